# revision 1
# baseline (speedup 1.0000x reference)
# Trainium2 Bass kernel for nn_MultiHeadAttentionPure (B=2, S=1024, F=1024, H=16).
#
# The reference splits q/k/v into 64 feature-chunks of 16 ("groups"), runs
# causal attention independently per (group, batch) pair -- 128 independent
# [1024,16] attention problems -- then applies a (buggy-but-faithful) torch
# reshape that scrambles (group, batch, seq) into the [B,S,F] tensor fed to
# the output linear layer.
#
# Sharding: the scramble maps output rows (b2, s2) to attention groups such
# that core c = b2*4 + q (q = s2_block of 256) needs exactly the 16 groups
# {j : j%4 == 2*b2 + q//2} at input batch b = q%2 -- a perfect partition of
# the 128 (group, batch) pairs across 8 cores with zero cross-core traffic.
# Each core computes its 16 attention groups, assembles its y^T tile
# ([1024 features, 256 rows]) on-chip, and runs the output linear for its
# 256 output rows.  Host slices inputs / concatenates outputs.
#
# On-device layout (per core, per group g):
#   scores^T[s2, s1] = k^T-slice (lhsT [16,128]) x q^T (rhs [16,512])  (fp32r)
#   causal mask: DVE adds -1e9 triangle onto PSUM before exp
#   expT = ACT Exp(PSUM) -> SBUF fp32r
#   x^T [33, s1] += vaug (lhsT [128,33], col 32 = ones) x expT   => row 32 = denom
#   recip = 1/denom (fp32r), PE-broadcast to 16 partitions via ones[1,16]
#   xs[h,m,r] = x^T[h, 4r+m] * recip  (DVE), DMA-scatter into y^T tile
#   out[r, o] = sum_f y^T[f,r] x W_out^T[f,o] + b_out  (fp32r matmuls)
import numpy as np

B, S, F, H = 2, 1024, 1024, 16
NG = 16          # groups per core
P = 128
NCORES = 8


def _fp32r(x):
    """Round fp32 -> fp32r (11-bit mantissa, round-half-up) like the HW expects."""
    b = np.ascontiguousarray(x.astype(np.float32)).view(np.uint32)
    r = ((b.astype(np.uint64) + 0x800) & 0xFFFFF000).astype(np.uint32)
    return r.view(np.float32)


def _core_groups(c):
    b2, qq = c // 4, c % 4
    b = qq % 2
    jmod = 2 * b2 + qq // 2
    js = [4 * h2 + jmod for h2 in range(NG)]
    return b2, qq, b, js


def _build(causal: bool, n_iter: int = 1):
    import concourse.bass as bass
    import concourse.mybir as mybir
    from concourse import bacc, tile

    F32 = mybir.dt.float32
    F32R = mybir.dt.float16   # attention/linear operand dtype (full-rate PE)
    AF = mybir.ActivationFunctionType
    ADD = mybir.AluOpType.add
    MUL = mybir.AluOpType.mult

    nc = bacc.Bacc("TRN2", target_bir_lowering=False, debug=False)
    qt = nc.declare_dram_parameter("qt", [NG * H, S], F32R, isOutput=False)
    kt = nc.declare_dram_parameter("kt", [NG * H, S], F32R, isOutput=False)
    va = nc.declare_dram_parameter("va", [S, NG * 33], F32R, isOutput=False)
    wt = nc.declare_dram_parameter("wt", [F, F], F32R, isOutput=False)
    msk = nc.declare_dram_parameter("msk", [P, 256], F32, isOutput=False)
    bb = nc.declare_dram_parameter("bb", [P, F], F32, isOutput=False)
    out = nc.declare_dram_parameter("o", [256, F], F32, isOutput=True)

    NT = S // P           # 8 s2 tiles
    NC_ = S // 512        # 2 s1 chunks

    import contextlib
    with tile.TileContext(nc) as tc:
        loop_ctx = tc.For_i(0, n_iter, 1, hint_engines=(
            mybir.EngineType.PE, mybir.EngineType.DVE, mybir.EngineType.Activation,
            mybir.EngineType.SP, mybir.EngineType.Pool,
        )) if n_iter > 1 else contextlib.nullcontext()
        with loop_ctx, \
             tc.tile_pool(name="cst", bufs=1) as cst, \
             tc.tile_pool(name="qk", bufs=4) as qkp, \
             tc.tile_pool(name="expp", bufs=3) as expp, \
             tc.tile_pool(name="work", bufs=3) as wkp, \
             tc.tile_pool(name="yt", bufs=1) as ytp, \
             tc.tile_pool(name="stps", bufs=5, space="PSUM") as stps, \
             tc.tile_pool(name="xtps", bufs=3, space="PSUM") as xtps:

            va_sb = cst.tile([P, NT, NG * 33], F32R)
            wt_sb = cst.tile([P, F // P, F], F32R)
            msk_sb = cst.tile([P, 256], F32)
            bb_sb = cst.tile([P, F], F32)
            nc.sync.dma_start(va_sb[:], va.rearrange("(t p) m -> p t m", p=P))
            nc.sync.dma_start(wt_sb[:], wt.rearrange("(t p) m -> p t m", p=P))
            nc.sync.dma_start(msk_sb[:], msk[:])
            nc.sync.dma_start(bb_sb[:], bb[:])

            yt_sb = ytp.tile([P, F // P, 256], F32R)

            for g in range(NG):
                qt_g = qkp.tile([H, S], F32R, tag="qt")
                kt_g = qkp.tile([H, S], F32R, tag="kt")
                nc.sync.dma_start(qt_g[:], qt[g * H:(g + 1) * H, :])
                nc.sync.dma_start(kt_g[:], kt[g * H:(g + 1) * H, :])
                for c in range(NC_):
                    ntile = 4 * c + 4 if causal else NT
                    expt = expp.tile([P, NT, 512], F32R, tag="expt")
                    xt = xtps.tile([33, 512], F32, tag="xt")
                    for t in range(ntile):
                        st = stps.tile([P, 512], F32, tag="st")
                        d = t - 4 * c
                        a1 = 128 * d if (causal and d >= 0) else 0
                        nc.tensor.matmul(
                            st[:, a1:], kt_g[:, t * P:(t + 1) * P],
                            qt_g[:, 512 * c + a1: 512 * (c + 1)],
                            start=True, stop=True)
                        if causal and d >= 0:
                            nc.vector.tensor_tensor(
                                out=st[:, a1:a1 + P], in0=st[:, a1:a1 + P],
                                in1=msk_sb[:, 128:], op=ADD)
                        nc.scalar.activation(expt[:, t, a1:], st[:, a1:], AF.Exp)
                        nc.tensor.matmul(
                            xt[:, a1:], va_sb[:, t, g * 33:(g + 1) * 33],
                            expt[:, t, a1:],
                            start=(t == 0), stop=(t == ntile - 1))
                    recip = wkp.tile([1, 512], F32R, tag="recip")
                    with nc.allow_low_precision(reason="fp16 softmax recip"):
                        nc.vector.reciprocal(recip[:], xt[32:33, :])
                    recipb = wkp.tile([16, 512], F32R, tag="recipb")
                    nc.gpsimd.partition_broadcast(recipb[:], recip[:])
                    xs = wkp.tile([16, 4, 128], F32R, tag="xs")
                    for m in range(4):
                        nc.vector.tensor_tensor(
                            out=xs[:, m, :], in0=xt[0:16, m:512:4],
                            in1=recipb[:, m:512:4], op=MUL)
                    po = 64 * (g % 2)
                    for m in range(4):
                        nc.sync.dma_start(
                            out=yt_sb[po + 16 * m: po + 16 * (m + 1), g // 2,
                                      128 * c:128 * (c + 1)],
                            in_=xs[:, m, :])

            # output linear: out[r, o] = sum_f yT[f, r] * wt[f, o] + b[o]
            for r2 in range(2):
                for oc in range(2):
                    ps_t = stps.tile([P, 512], F32, tag="st")
                    ps = ps_t[:]
                    for ft in range(F // P):
                        nc.tensor.matmul(
                            ps, yt_sb[:, ft, r2 * P:(r2 + 1) * P],
                            wt_sb[:, ft, oc * 512:(oc + 1) * 512],
                            start=(ft == 0), stop=(ft == F // P - 1))
                    ot = wkp.tile([P, 512], F32, tag="ot")
                    nc.vector.tensor_tensor(
                        out=ot[:], in0=ps, in1=bb_sb[:, oc * 512:(oc + 1) * 512],
                        op=ADD)
                    nc.sync.dma_start(
                        out=out[r2 * P:(r2 + 1) * P, oc * 512:(oc + 1) * 512],
                        in_=ot[:])
    nc.compile()
    return nc


_NC_CACHE = {}


def _get_nc(causal: bool, n_iter: int = 1):
    key = (causal, n_iter)
    if key not in _NC_CACHE:
        _NC_CACHE[key] = _build(causal, n_iter)
    return _NC_CACHE[key]


def _shard_inputs(q, k, v, W_out, b_out):
    """Build the 8 per-core input maps (all fp32r pre-rounded where needed)."""
    wt = np.ascontiguousarray(W_out.T).astype(np.float16)
    mskv = np.full((P, 256), -1e9, np.float32)
    xi, yi = np.mgrid[0:P, 0:P]
    mskv[:, 128:] = np.where(yi >= xi, 0.0, -1e9).astype(np.float32)
    bbv = np.broadcast_to(b_out.astype(np.float32), (P, F)).copy()

    in_maps = []
    for c in range(NCORES):
        _, _, b, js = _core_groups(c)
        cols = np.concatenate([j * H + np.arange(H) for j in js])
        qc = (0.25 * q[b][:, cols].T).astype(np.float16)     # [256, S]
        kc = np.ascontiguousarray(k[b][:, cols].T).astype(np.float16)
        vav = np.zeros((S, NG, 33), np.float32)
        vav[:, :, :16] = v[b][:, cols].reshape(S, NG, H)
        vav[:, :, 32] = 1.0
        in_maps.append({
            "qt": np.ascontiguousarray(qc),
            "kt": kc,
            "va": vav.reshape(S, NG * 33).astype(np.float16),
            "wt": wt,
            "msk": mskv,
            "bb": bbv,
        })
    return in_maps


def _unshard(outs):
    full = np.empty((B, S, F), np.float32)
    for c in range(NCORES):
        b2, qq, _, _ = _core_groups(c)
        full[b2, 256 * qq:256 * (qq + 1), :] = outs[c]
    return full


def _numpy_core(in_map, causal=True):
    """Numpy emulation of the device program (for host-logic validation)."""
    qt = in_map["qt"].astype(np.float32); kt = in_map["kt"].astype(np.float32)
    va = in_map["va"].reshape(S, NG, 33).astype(np.float32)
    wtm = in_map["wt"].astype(np.float32); bbv = in_map["bb"]
    ytv = np.zeros((F, 256), np.float32)
    for g in range(NG):
        sc = kt[g * H:(g + 1) * H].T @ qt[g * H:(g + 1) * H]   # [s2, s1]
        if causal:
            s2i, s1i = np.mgrid[0:S, 0:S]
            sc = np.where(s1i >= s2i, sc, -1e9)
        e = np.exp(sc).astype(np.float16).astype(np.float32)
        if causal:
            e = np.where(s1i >= s2i, e, 0.0).astype(np.float32)
        xt = va[:, g, :].T @ e                                  # [33, s1]
        recip = (1.0 / xt[32]).astype(np.float16).astype(np.float32)
        xs = (xt[0:16] * recip[None, :]).astype(np.float16).astype(np.float32)                  # [h, s1]
        po = 64 * (g % 2)
        for m in range(4):
            for cc in range(2):
                ytv[128 * (g // 2) + po + 16 * m: 128 * (g // 2) + po + 16 * (m + 1),
                    128 * cc:128 * (cc + 1)] = xs[:, 512 * cc + m:512 * (cc + 1):4]
    o = ytv.T @ wtm + bbv[0][None, :]
    return o.astype(np.float32)


def kernel(q, k, v, W_out, b_out, apply_mask, _mock=False):
    q = np.asarray(q, np.float32)
    k = np.asarray(k, np.float32)
    v = np.asarray(v, np.float32)
    W_out = np.asarray(W_out, np.float32)
    b_out = np.asarray(b_out, np.float32)
    causal = bool(int(np.asarray(apply_mask)))
    in_maps = _shard_inputs(q, k, v, W_out, b_out)
    if _mock:
        outs = [_numpy_core(m, causal) for m in in_maps]
        return _unshard(outs)
    from concourse.bass_utils import run_bass_kernel_spmd
    nc = _get_nc(causal)
    res = run_bass_kernel_spmd(nc, in_maps, core_ids=list(range(NCORES)))
    return _unshard([r["o"] for r in res.results])



# revision 13
# speedup vs baseline: 1.1854x; 1.1854x over previous
# Trainium2 Bass kernel for nn_MultiHeadAttentionPure (B=2, S=1024, F=1024, H=16).
#
# The reference splits q/k/v into 64 feature-chunks of 16 ("groups"), runs
# causal attention independently per (group, batch) pair -- 128 independent
# [1024,16] attention problems -- then applies a (buggy-but-faithful) torch
# reshape that scrambles (group, batch, seq) into the [B,S,F] tensor fed to
# the output linear layer.
#
# Sharding: core c = b2*4 + qq (qq = s'-block of 256) needs exactly the 16
# groups {j : j%4 == 2*b2 + qq//2} at input batch b = qq%2 -- a perfect
# partition of the 128 (group, batch) pairs across 8 cores with zero
# cross-core traffic.  Output row (b', s') feature f = 64*g' + 16*m + h maps
# to local group g', seq s = 4*u + m (u = s' mod 256), head h -- so the
# output linear is computed transposed (out^T = W_perm^T @ xs) with stride-4
# moving-operand access patterns doing the m-deinterleave for free.
#
# On-device layout (per core, per quartet q of 4 groups j=0..3):
#   scores^T st[s2,s1] = kt-slice (lhsT [16,128] @ rows 32j) x qt (row-tiled
#     PE position (32j, 0)) -> PSUM [128, 1024] (2 banks), trimmed causally
#   expt = ACT Exp over [128, 128t:1024] (one instr per (g,t)) -> SBUF fp16
#   diag-block mask: DVE fp16 multiply with 0/1 upper-tri [128,128]
#   xt[32j:32j+32, s1] += va (lhsT [128,32] = [v|ones|zeros]) x expt
#     (col-tiled PE position (0, 32j), 4 groups share one PSUM bank)
#     -> row 32j+16 = denom, rows 32j+17.. = 0
#   recip = DVE reciprocal of whole bank -> fp16; gpsimd broadcasts the denom
#     rows to the x rows of recipb (junk rows memset 0 once)
#   xs[:, q, s1] = xt * recipb (one DVE op; junk rows exact 0)
#   out^T[o,r] = sum_{q,m} wtp[(q,m)][128,128] x xs[:, q, m::4]  (N=256)
#   bias add per-partition (DVE tensor_scalar), DMA out^T [1024, 256].
import numpy as np

B, S, F, H = 2, 1024, 1024, 16
NG = 16          # groups per core
P = 128
NCORES = 8
NT = S // P      # 8 s2 tiles


def _core_groups(c):
    b2, qq = c // 4, c % 4
    b = qq % 2
    jmod = 2 * b2 + qq // 2
    js = [4 * h2 + jmod for h2 in range(NG)]
    return b2, qq, b, js


def _build(causal: bool, n_iter: int = 1):
    import concourse.bass as bass
    import concourse.mybir as mybir
    from concourse import bacc, tile

    F32 = mybir.dt.float32
    F16 = mybir.dt.float16
    AF = mybir.ActivationFunctionType
    MUL = mybir.AluOpType.mult
    ADD = mybir.AluOpType.add

    nc = bacc.Bacc("TRN2", target_bir_lowering=False, debug=False)
    ktp = nc.declare_dram_parameter("ktp", [P, 4 * S], F16, isOutput=False)
    qtp = nc.declare_dram_parameter("qtp", [P, 4 * S], F16, isOutput=False)
    vap = nc.declare_dram_parameter("vap", [P, NT * 4 * P], F16, isOutput=False)
    wtp = nc.declare_dram_parameter("wtp", [P, 16 * F], F16, isOutput=False)
    bop = nc.declare_dram_parameter("bop", [P, 8], F32, isOutput=False)
    mkp = nc.declare_dram_parameter("mkp", [P, P], F16, isOutput=False)
    out = nc.declare_dram_parameter("o", [F, 256], F32, isOutput=True)

    import contextlib
    with tile.TileContext(nc) as tc:
        loop_ctx = tc.For_i(0, n_iter, 1, hint_engines=(
            mybir.EngineType.PE, mybir.EngineType.DVE, mybir.EngineType.Activation,
            mybir.EngineType.SP, mybir.EngineType.Pool,
        )) if n_iter > 1 else contextlib.nullcontext()
        with loop_ctx, \
             tc.tile_pool(name="cst", bufs=1) as cst, \
             tc.tile_pool(name="expp", bufs=6) as expp, \
             tc.tile_pool(name="work", bufs=3) as wkp, \
             tc.tile_pool(name="stps", bufs=2, space="PSUM") as stps, \
             tc.tile_pool(name="xtps", bufs=2, space="PSUM") as xtps:

            kt_sb = cst.tile([P, 4, S], F16)
            qt_sb = cst.tile([P, 4, S], F16)
            va_sb = cst.tile([P, NT, 4 * P], F16)
            wt_sb = cst.tile([P, 16, F], F16)
            bo_sb = cst.tile([P, 8], F32)
            mk_sb = cst.tile([P, P], F16)
            on_sb = cst.tile([P, 32], F16)
            recipb = cst.tile([P, 2, 512], F16)
            xs_sb = cst.tile([P, 4, S], F16)
            nc.sync.dma_start(kt_sb[:], ktp[:])
            nc.sync.dma_start(qt_sb[:], qtp[:])
            nc.sync.dma_start(va_sb[:], vap[:])
            nc.sync.dma_start(wt_sb[:], wtp[:])
            nc.sync.dma_start(bo_sb[:], bop[:])
            nc.sync.dma_start(mk_sb[:], mkp[:])
            nc.vector.memset(on_sb[:], 1.0)

            for q in range(4):
                xtb = xtps.tile([P, 2, 512], F32, tag="xt")
                for pair in range(2):
                    for t in range(NT):
                        for j2 in range(2):
                            j = 2 * pair + j2
                            g = 4 * q + j
                            st = stps.tile([P, S], F32, tag="st")
                            t0 = 128 * t if causal else 0
                            # scores, per 512-col chunk (PSUM bank)
                            for c in range(2):
                                lo = max(t0, 512 * c)
                                if lo >= 512 * (c + 1):
                                    continue
                                nc.tensor.matmul(
                                    st[:, lo:512 * (c + 1)],
                                    kt_sb[32 * j:32 * j + 16, q, 128 * t:128 * (t + 1)],
                                    qt_sb[32 * j:32 * j + 16, q, lo:512 * (c + 1)],
                                    start=True, stop=True,
                                    tile_position=(32 * j, 0))
                            expt = expp.tile([P, S], F16, tag="expt")
                            nc.scalar.activation(expt[:, t0:], st[:, t0:], AF.Exp)
                            if causal:
                                nc.vector.tensor_tensor(
                                    out=expt[:, t0:t0 + P], in0=expt[:, t0:t0 + P],
                                    in1=mk_sb[:], op=MUL)
                            # attn @ [v|ones|zeros], col-tiled into shared bank
                            for c in range(2):
                                lo = max(t0, 512 * c)
                                if lo >= 512 * (c + 1):
                                    continue
                                last_t = 3 if (causal and c == 0) else NT - 1
                                nc.tensor.matmul(
                                    xtb[32 * j:32 * (j + 1), c, lo - 512 * c:],
                                    va_sb[:, t, 32 * g:32 * (g + 1)],
                                    expt[:, lo:512 * (c + 1)],
                                    start=(t == 0), stop=(t == last_t),
                                    skip_group_check=True,
                                    tile_position=(0, 32 * j))
                for c in range(2):
                    recip_all = wkp.tile([P, 512], F16, tag="recip")
                    with nc.allow_low_precision(reason="fp16 softmax recip"):
                        nc.vector.reciprocal(recip_all[:], xtb[:, c, :])
                    # PE broadcast: denom-recip row 32j -> whole 32-block
                    rb_ps = stps.tile([P, 512], F32, tag="st")
                    for j in range(4):
                        nc.tensor.matmul(
                            rb_ps[32 * j:32 * (j + 1), :],
                            on_sb[32 * j:32 * j + 1, :],
                            recip_all[32 * j:32 * j + 1, :],
                            start=True, stop=True,
                            tile_position=(32 * j, 32 * j))
                    nc.vector.tensor_scalar(
                        out=recipb[:, c, :], in0=rb_ps[:],
                        scalar1=0.0, scalar2=None, op0=ADD)
                    nc.vector.tensor_tensor(
                        out=xs_sb[:, q, 512 * c:512 * (c + 1)],
                        in0=xtb[:, c, :], in1=recipb[:, c, :], op=MUL)

            # output linear: out^T[o, r] = sum_{q,m} wtp_(q,m)^T @ xs[:, q, m::4]
            for ot in range(8):
                ps = stps.tile([P, 256], F32, tag="st")
                for q in range(4):
                    for m in range(4):
                        nc.tensor.matmul(
                            ps[:], wt_sb[:, 4 * q + m, 128 * ot:128 * (ot + 1)],
                            xs_sb[:, q, m:S:4],
                            start=(q == 0 and m == 0), stop=(q == 3 and m == 3))
                ot_sb = wkp.tile([P, 256], F32, tag="ot")
                nc.vector.tensor_scalar(
                    out=ot_sb[:], in0=ps[:], scalar1=bo_sb[:, ot:ot + 1],
                    scalar2=None, op0=ADD)
                nc.sync.dma_start(out=out[128 * ot:128 * (ot + 1), :], in_=ot_sb[:])
    nc.compile()
    return nc


_NC_CACHE = {}


def _get_nc(causal: bool, n_iter: int = 1):
    key = (causal, n_iter)
    if key not in _NC_CACHE:
        _NC_CACHE[key] = _build(causal, n_iter)
    return _NC_CACHE[key]


def _shard_inputs(q, k, v, W_out, b_out):
    """Build the 8 per-core input maps."""
    mk = (np.arange(P)[None, :] >= np.arange(P)[:, None]).astype(np.float16)

    in_maps = []
    for c in range(NCORES):
        _, _, b, js = _core_groups(c)
        cols = np.concatenate([j * H + np.arange(H) for j in js])  # [NG*H]
        kv = k[b][:, cols]          # [S, 256] cols grouped by local g'
        qv = q[b][:, cols] * 0.25
        vv = v[b][:, cols]

        ktm = np.zeros((P, 4 * S), np.float16)
        qtm = np.zeros((P, 4 * S), np.float16)
        for g2 in range(NG):
            qq_, j = divmod(g2, 4)
            rows = slice(32 * j, 32 * j + 16)
            csl = slice(S * qq_, S * (qq_ + 1))
            ktm[rows, csl] = kv[:, 16 * g2:16 * (g2 + 1)].T.astype(np.float16)
            qtm[rows, csl] = qv[:, 16 * g2:16 * (g2 + 1)].T.astype(np.float16)

        vam = np.zeros((P, NT, 4 * P), np.float16)
        for t in range(NT):
            blk = vv[128 * t:128 * (t + 1)].reshape(P, NG, H)  # [p, g', h]
            for g2 in range(NG):
                vam[:, t, 32 * g2] = 1.0
                vam[:, t, 32 * g2 + 1:32 * g2 + 17] = blk[:, g2].astype(np.float16)

        wtm = np.zeros((P, 16 * F), np.float16)
        for q4 in range(4):
            for m in range(4):
                ft = 4 * q4 + m
                for j in range(4):
                    g2 = 4 * q4 + j
                    fidx = 64 * g2 + 16 * m + np.arange(H)
                    wtm[32 * j + 1:32 * j + 17, F * ft:F * (ft + 1)] = \
                        W_out[:, fidx].T.astype(np.float16)

        bom = np.ascontiguousarray(
            b_out.astype(np.float32).reshape(8, P).T)  # [p, ot]

        in_maps.append({
            "ktp": ktm, "qtp": qtm,
            "vap": vam.reshape(P, NT * 4 * P),
            "wtp": wtm, "bop": bom, "mkp": mk,
        })
    return in_maps


def _unshard(outs):
    full = np.empty((B, S, F), np.float32)
    for c in range(NCORES):
        b2, qq, _, _ = _core_groups(c)
        full[b2, 256 * qq:256 * (qq + 1), :] = outs[c].T
    return full


def _numpy_core(in_map, causal=True):
    """Numpy emulation of the device program (for host-logic validation)."""
    ktm = in_map["ktp"].astype(np.float32)
    qtm = in_map["qtp"].astype(np.float32)
    vam = in_map["vap"].reshape(P, NT, 4 * P).astype(np.float32)
    wtm = in_map["wtp"].astype(np.float32)
    bom = in_map["bop"]

    xs = np.zeros((P, 4, S), np.float32)
    for q4 in range(4):
        xt = np.zeros((P, S), np.float32)
        for j in range(4):
            g2 = 4 * q4 + j
            kt = ktm[32 * j:32 * j + 16, S * q4:S * (q4 + 1)]
            qt = qtm[32 * j:32 * j + 16, S * q4:S * (q4 + 1)]
            st = kt.T @ qt                      # [s2, s1]
            e = np.exp(st).astype(np.float16).astype(np.float32)
            if causal:
                s2i, s1i = np.mgrid[0:S, 0:S]
                e = np.where(s1i >= s2i, e, 0.0)
            va = np.concatenate([vam[:, t, :] for t in range(NT)], axis=0)  # [S, 4P]
            xt[32 * j:32 * (j + 1), :] = va[:, 32 * g2:32 * (g2 + 1)].T @ e
        recip_all = np.float16(1.0) / xt.astype(np.float16)
        recipb = np.zeros((P, S), np.float16)
        for j in range(4):
            recipb[32 * j:32 * (j + 1)] = recip_all[32 * j]
        xs[:, q4, :] = (xt * recipb.astype(np.float32)).astype(np.float16)

    oT = np.zeros((F, 256), np.float32)
    for ot in range(8):
        acc = np.zeros((P, 256), np.float32)
        for q4 in range(4):
            for m in range(4):
                lhsT = wtm[:, F * (4 * q4 + m) + 128 * ot:F * (4 * q4 + m) + 128 * (ot + 1)]
                rhs = xs[:, q4, m::4]
                acc += lhsT.T @ rhs
        oT[128 * ot:128 * (ot + 1)] = acc + bom[:, ot][:, None]
    return oT


def kernel(q, k, v, W_out, b_out, apply_mask, _mock=False):
    q = np.asarray(q, np.float32)
    k = np.asarray(k, np.float32)
    v = np.asarray(v, np.float32)
    W_out = np.asarray(W_out, np.float32)
    b_out = np.asarray(b_out, np.float32)
    causal = bool(int(np.asarray(apply_mask)))
    in_maps = _shard_inputs(q, k, v, W_out, b_out)
    if _mock:
        outs = [_numpy_core(m, causal) for m in in_maps]
        return _unshard(outs)
    from concourse.bass_utils import run_bass_kernel_spmd
    nc = _get_nc(causal)
    res = run_bass_kernel_spmd(nc, in_maps, core_ids=list(range(NCORES)))
    return _unshard([r["o"] for r in res.results])


# revision 17
# speedup vs baseline: 1.2343x; 1.0412x over previous
# Trainium2 Bass kernel for nn_MultiHeadAttentionPure (B=2, S=1024, F=1024, H=16).
#
# The reference splits q/k/v into 64 feature-chunks of 16 ("groups"), runs
# causal attention independently per (group, batch) pair -- 128 independent
# [1024,16] attention problems -- then applies a (buggy-but-faithful) torch
# reshape that scrambles (group, batch, seq) into the [B,S,F] tensor fed to
# the output linear layer.
#
# Sharding: core c = b2*4 + qq (qq = s'-block of 256) needs exactly the 16
# groups {j : j%4 == 2*b2 + qq//2} at input batch b = qq%2 -- a perfect
# partition of the 128 (group, batch) pairs across 8 cores with zero
# cross-core traffic.  Output row (b', s') feature f = 64*g' + 16*m + h maps
# to local group g', seq s = 4*u + m (u = s' mod 256), head h -- so the
# output linear is computed transposed (out^T = W_perm^T @ xs) with stride-4
# moving-operand access patterns doing the m-deinterleave for free.
#
# On-device layout (per core, per quartet q of 4 groups j=0..3):
#   scores^T st[s2,s1] = kt-slice (lhsT [16,128] @ rows 32j) x qt (row-tiled
#     PE position (32j, 0)) -> PSUM [128, 1024] (2 banks), trimmed causally
#   expt = ACT Exp over [128, 128t:1024] (one instr per (g,t)) -> SBUF fp16
#   diag-block mask: DVE fp16 multiply with 0/1 upper-tri [128,128]
#   xt[32j:32j+32, s1] += va (lhsT [128,32] = [v|ones|zeros]) x expt
#     (col-tiled PE position (0, 32j), 4 groups share one PSUM bank)
#     -> row 32j+16 = denom, rows 32j+17.. = 0
#   recip = DVE reciprocal of whole bank -> fp16; gpsimd broadcasts the denom
#     rows to the x rows of recipb (junk rows memset 0 once)
#   xs[:, q, s1] = xt * recipb (one DVE op; junk rows exact 0)
#   out^T[o,r] = sum_{q,m} wtp[(q,m)][128,128] x xs[:, q, m::4]  (N=256)
#   bias add per-partition (DVE tensor_scalar), DMA out^T [1024, 256].
import numpy as np

B, S, F, H = 2, 1024, 1024, 16
NG = 16          # groups per core
P = 128
NCORES = 8
NT = S // P      # 8 s2 tiles


def _core_groups(c):
    b2, qq = c // 4, c % 4
    b = qq % 2
    jmod = 2 * b2 + qq // 2
    js = [4 * h2 + jmod for h2 in range(NG)]
    return b2, qq, b, js


def _build(causal: bool, n_iter: int = 1):
    import concourse.bass as bass
    import concourse.mybir as mybir
    from concourse import bacc, tile

    F32 = mybir.dt.float32
    F16 = mybir.dt.float16
    AF = mybir.ActivationFunctionType
    MUL = mybir.AluOpType.mult
    ADD = mybir.AluOpType.add

    nc = bacc.Bacc("TRN2", target_bir_lowering=False, debug=False)
    ktp = nc.declare_dram_parameter("ktp", [P, 4 * S], F16, isOutput=False)
    qtp = nc.declare_dram_parameter("qtp", [P, 4 * S], F16, isOutput=False)
    vap = nc.declare_dram_parameter("vap", [P, NT * 4 * P], F16, isOutput=False)
    wtp = nc.declare_dram_parameter("wtp", [P, 16 * F], F16, isOutput=False)
    bop = nc.declare_dram_parameter("bop", [P, 8], F32, isOutput=False)
    mkp = nc.declare_dram_parameter("mkp", [P, P], F16, isOutput=False)
    out = nc.declare_dram_parameter("o", [F, 256], F32, isOutput=True)

    import contextlib
    with tile.TileContext(nc) as tc:
        loop_ctx = tc.For_i(0, n_iter, 1, hint_engines=(
            mybir.EngineType.PE, mybir.EngineType.DVE, mybir.EngineType.Activation,
            mybir.EngineType.SP, mybir.EngineType.Pool,
        )) if n_iter > 1 else contextlib.nullcontext()
        with loop_ctx, \
             tc.tile_pool(name="cst", bufs=1) as cst, \
             tc.tile_pool(name="expp", bufs=6) as expp, \
             tc.tile_pool(name="work", bufs=3) as wkp, \
             tc.tile_pool(name="stps", bufs=2, space="PSUM") as stps, \
             tc.tile_pool(name="rbps", bufs=2, space="PSUM") as rbps, \
             tc.tile_pool(name="xtps", bufs=1, space="PSUM") as xtps:

            kt_sb = cst.tile([P, 4, S], F16)
            qt_sb = cst.tile([P, 4, S], F16)
            va_sb = cst.tile([P, NT, 4 * P], F16)
            wt_sb = cst.tile([P, 16, F], F16)
            bo_sb = cst.tile([P, 8], F32)
            mk_sb = cst.tile([P, P], F16)
            on_sb = cst.tile([P, 32], F16)
            recipb = cst.tile([P, 2, 512], F16)
            xs_sb = cst.tile([P, 4, S], F16)
            oacc = cst.tile([P, 8, 256], F32)
            nc.sync.dma_start(kt_sb[:, 0, :], ktp[:, 0:S])
            nc.sync.dma_start(qt_sb[:, 0, :], qtp[:, 0:S])
            nc.sync.dma_start(mk_sb[:], mkp[:])
            nc.sync.dma_start(va_sb[:], vap[:])
            nc.sync.dma_start(bo_sb[:], bop[:])
            for qd in range(1, 4):
                nc.sync.dma_start(kt_sb[:, qd, :], ktp[:, S * qd:S * (qd + 1)])
                nc.sync.dma_start(qt_sb[:, qd, :], qtp[:, S * qd:S * (qd + 1)])
            for fd in range(4):
                nc.sync.dma_start(wt_sb[:, 4 * fd:4 * (fd + 1), :],
                                  wtp[:, 4 * F * fd:4 * F * (fd + 1)])
            nc.vector.memset(on_sb[:], 1.0)

            for q in range(4):
                xtb = xtps.tile([P, 2, 512], F32, tag="xt")
                for pair in range(2):
                    for t in range(NT):
                        for j2 in range(2):
                            j = 2 * pair + j2
                            g = 4 * q + j
                            st = stps.tile([P, S], F32, tag="st")
                            t0 = 128 * t if causal else 0
                            # scores, per 512-col chunk (PSUM bank)
                            for c in range(2):
                                lo = max(t0, 512 * c)
                                if lo >= 512 * (c + 1):
                                    continue
                                nc.tensor.matmul(
                                    st[:, lo:512 * (c + 1)],
                                    kt_sb[32 * j:32 * j + 16, q, 128 * t:128 * (t + 1)],
                                    qt_sb[32 * j:32 * j + 16, q, lo:512 * (c + 1)],
                                    start=True, stop=True,
                                    tile_position=(32 * j, 0))
                            expt = expp.tile([P, S], F16, tag="expt")
                            nc.scalar.activation(expt[:, t0:], st[:, t0:], AF.Exp)
                            if causal:
                                nc.vector.tensor_tensor(
                                    out=expt[:, t0:t0 + P], in0=expt[:, t0:t0 + P],
                                    in1=mk_sb[:], op=MUL)
                            # attn @ [v|ones|zeros], col-tiled into shared bank
                            for c in range(2):
                                lo = max(t0, 512 * c)
                                if lo >= 512 * (c + 1):
                                    continue
                                last_t = 3 if (causal and c == 0) else NT - 1
                                nc.tensor.matmul(
                                    xtb[32 * j:32 * (j + 1), c, lo - 512 * c:],
                                    va_sb[:, t, 32 * g:32 * (g + 1)],
                                    expt[:, lo:512 * (c + 1)],
                                    start=(t == 0), stop=(t == last_t),
                                    skip_group_check=True,
                                    tile_position=(0, 32 * j))
                for c in range(2):
                    recip_all = wkp.tile([P, 512], F16, tag="recip")
                    with nc.allow_low_precision(reason="fp16 softmax recip"):
                        nc.vector.reciprocal(recip_all[:], xtb[:, c, :])
                    # PE broadcast: denom-recip row 32j -> whole 32-block
                    rb_ps = rbps.tile([P, 512], F32, tag="rb")
                    for j in range(4):
                        nc.tensor.matmul(
                            rb_ps[32 * j:32 * (j + 1), :],
                            on_sb[32 * j:32 * j + 1, :],
                            recip_all[32 * j:32 * j + 1, :],
                            start=True, stop=True,
                            tile_position=(32 * j, 32 * j))
                    nc.vector.tensor_scalar(
                        out=recipb[:, c, :], in0=rb_ps[:],
                        scalar1=0.0, scalar2=None, op0=ADD)
                    nc.vector.tensor_tensor(
                        out=xs_sb[:, q, 512 * c:512 * (c + 1)],
                        in0=xtb[:, c, :], in1=recipb[:, c, :], op=MUL)

                # this quartet's share of the output linear:
                # out^T[o, r] += sum_m wtp_(q,m)^T @ xs[:, q, m::4]
                for ot in range(8):
                    ps = rbps.tile([P, 256], F32, tag="rb")
                    for m in range(4):
                        nc.tensor.matmul(
                            ps[:], wt_sb[:, 4 * q + m, 128 * ot:128 * (ot + 1)],
                            xs_sb[:, q, m:S:4],
                            start=(m == 0), stop=(m == 3))
                    if q == 0:
                        nc.vector.tensor_scalar(
                            out=oacc[:, ot, :], in0=ps[:],
                            scalar1=bo_sb[:, ot:ot + 1], scalar2=None, op0=ADD)
                    else:
                        nc.vector.tensor_tensor(
                            out=oacc[:, ot, :], in0=ps[:], in1=oacc[:, ot, :],
                            op=ADD)
                    if q == 3:
                        nc.sync.dma_start(out=out[128 * ot:128 * (ot + 1), :],
                                          in_=oacc[:, ot, :])
    nc.compile()
    return nc


_NC_CACHE = {}


def _get_nc(causal: bool, n_iter: int = 1):
    key = (causal, n_iter)
    if key not in _NC_CACHE:
        _NC_CACHE[key] = _build(causal, n_iter)
    return _NC_CACHE[key]


def _shard_inputs(q, k, v, W_out, b_out):
    """Build the 8 per-core input maps."""
    mk = (np.arange(P)[None, :] >= np.arange(P)[:, None]).astype(np.float16)

    in_maps = []
    for c in range(NCORES):
        _, _, b, js = _core_groups(c)
        cols = np.concatenate([j * H + np.arange(H) for j in js])  # [NG*H]
        kv = k[b][:, cols]          # [S, 256] cols grouped by local g'
        qv = q[b][:, cols] * 0.25
        vv = v[b][:, cols]

        ktm = np.zeros((P, 4 * S), np.float16)
        qtm = np.zeros((P, 4 * S), np.float16)
        for g2 in range(NG):
            qq_, j = divmod(g2, 4)
            rows = slice(32 * j, 32 * j + 16)
            csl = slice(S * qq_, S * (qq_ + 1))
            ktm[rows, csl] = kv[:, 16 * g2:16 * (g2 + 1)].T.astype(np.float16)
            qtm[rows, csl] = qv[:, 16 * g2:16 * (g2 + 1)].T.astype(np.float16)

        vam = np.zeros((P, NT, 4 * P), np.float16)
        for t in range(NT):
            blk = vv[128 * t:128 * (t + 1)].reshape(P, NG, H)  # [p, g', h]
            for g2 in range(NG):
                vam[:, t, 32 * g2] = 1.0
                vam[:, t, 32 * g2 + 1:32 * g2 + 17] = blk[:, g2].astype(np.float16)

        wtm = np.zeros((P, 16 * F), np.float16)
        for q4 in range(4):
            for m in range(4):
                ft = 4 * q4 + m
                for j in range(4):
                    g2 = 4 * q4 + j
                    fidx = 64 * g2 + 16 * m + np.arange(H)
                    wtm[32 * j + 1:32 * j + 17, F * ft:F * (ft + 1)] = \
                        W_out[:, fidx].T.astype(np.float16)

        bom = np.ascontiguousarray(
            b_out.astype(np.float32).reshape(8, P).T)  # [p, ot]

        in_maps.append({
            "ktp": ktm, "qtp": qtm,
            "vap": vam.reshape(P, NT * 4 * P),
            "wtp": wtm, "bop": bom, "mkp": mk,
        })
    return in_maps


def _unshard(outs):
    full = np.empty((B, S, F), np.float32)
    for c in range(NCORES):
        b2, qq, _, _ = _core_groups(c)
        full[b2, 256 * qq:256 * (qq + 1), :] = outs[c].T
    return full


def _numpy_core(in_map, causal=True):
    """Numpy emulation of the device program (for host-logic validation)."""
    ktm = in_map["ktp"].astype(np.float32)
    qtm = in_map["qtp"].astype(np.float32)
    vam = in_map["vap"].reshape(P, NT, 4 * P).astype(np.float32)
    wtm = in_map["wtp"].astype(np.float32)
    bom = in_map["bop"]

    xs = np.zeros((P, 4, S), np.float32)
    for q4 in range(4):
        xt = np.zeros((P, S), np.float32)
        for j in range(4):
            g2 = 4 * q4 + j
            kt = ktm[32 * j:32 * j + 16, S * q4:S * (q4 + 1)]
            qt = qtm[32 * j:32 * j + 16, S * q4:S * (q4 + 1)]
            st = kt.T @ qt                      # [s2, s1]
            e = np.exp(st).astype(np.float16).astype(np.float32)
            if causal:
                s2i, s1i = np.mgrid[0:S, 0:S]
                e = np.where(s1i >= s2i, e, 0.0)
            va = np.concatenate([vam[:, t, :] for t in range(NT)], axis=0)  # [S, 4P]
            xt[32 * j:32 * (j + 1), :] = va[:, 32 * g2:32 * (g2 + 1)].T @ e
        recip_all = np.float16(1.0) / xt.astype(np.float16)
        recipb = np.zeros((P, S), np.float16)
        for j in range(4):
            recipb[32 * j:32 * (j + 1)] = recip_all[32 * j]
        xs[:, q4, :] = (xt * recipb.astype(np.float32)).astype(np.float16)

    oT = np.zeros((F, 256), np.float32)
    for ot in range(8):
        acc = np.zeros((P, 256), np.float32)
        for q4 in range(4):
            for m in range(4):
                lhsT = wtm[:, F * (4 * q4 + m) + 128 * ot:F * (4 * q4 + m) + 128 * (ot + 1)]
                rhs = xs[:, q4, m::4]
                acc += lhsT.T @ rhs
        oT[128 * ot:128 * (ot + 1)] = acc + bom[:, ot][:, None]
    return oT


def kernel(q, k, v, W_out, b_out, apply_mask, _mock=False):
    q = np.asarray(q, np.float32)
    k = np.asarray(k, np.float32)
    v = np.asarray(v, np.float32)
    W_out = np.asarray(W_out, np.float32)
    b_out = np.asarray(b_out, np.float32)
    causal = bool(int(np.asarray(apply_mask)))
    in_maps = _shard_inputs(q, k, v, W_out, b_out)
    if _mock:
        outs = [_numpy_core(m, causal) for m in in_maps]
        return _unshard(outs)
    from concourse.bass_utils import run_bass_kernel_spmd
    nc = _get_nc(causal)
    res = run_bass_kernel_spmd(nc, in_maps, core_ids=list(range(NCORES)))
    return _unshard([r["o"] for r in res.results])


# revision 22
# speedup vs baseline: 7.1388x; 5.7839x over previous
# Trainium2 Bass kernel for nn_MultiHeadAttentionPure (B=2, S=1024, F=1024, H=16).
#
# The reference splits q/k/v into 64 feature-chunks of 16 ("groups"), runs
# causal attention independently per (group, batch) pair -- 128 independent
# [1024,16] attention problems -- then applies a (buggy-but-faithful) torch
# reshape that scrambles (group, batch, seq) into the [B,S,F] tensor fed to
# the output linear layer.
#
# Sharding: core c = b2*4 + qq (qq = s'-block of 256) needs exactly the 16
# groups {j : j%4 == 2*b2 + qq//2} at input batch b = qq%2 -- a perfect
# partition of the 128 (group, batch) pairs across 8 cores with zero
# cross-core traffic.  Output row (b', s') feature f = 64*g' + 16*m + h maps
# to local group g', seq s = 4*u + m (u = s' mod 256), head h -- so the
# output linear is computed transposed (out^T = W_perm^T @ xs) with stride-4
# moving-operand access patterns doing the m-deinterleave for free.
#
# On-device layout (per core, per quartet q of 4 groups j=0..3):
#   scores^T st[s2,s1] = kt-slice (lhsT [16,128] @ rows 32j) x qt (row-tiled
#     PE position (32j, 0)) -> PSUM [128, 1024] (2 banks), trimmed causally
#   expt = ACT Exp over [128, 128t:1024] (one instr per (g,t)) -> SBUF fp16
#   diag-block mask: DVE fp16 multiply with 0/1 upper-tri [128,128]
#   xt[32j:32j+32, s1] += va (lhsT [128,32] = [v|ones|zeros]) x expt
#     (col-tiled PE position (0, 32j), 4 groups share one PSUM bank)
#     -> row 32j+16 = denom, rows 32j+17.. = 0
#   denom at row 32j (va = [ones|v|zeros]): DVE reciprocal of whole bank ->
#     fp16; PE K=1 matmuls (ones[1,32] x recip-row) broadcast it across each
#     32-block; one DVE copy PSUM->SBUF; junk rows multiply to exact 0
#   xs[:, q, s1] = xt * recipb (one DVE op)
#   out^T[o,r] = sum_{q,m} wtp[(q,m)][128,128] x xs[:, q, m::4]  (N=256)
#   bias add per-partition (DVE tensor_scalar), DMA out^T [1024, 256].
import numpy as np

B, S, F, H = 2, 1024, 1024, 16
NG = 16          # groups per core
P = 128
NCORES = 8
NT = S // P      # 8 s2 tiles


def _core_groups(c):
    b2, qq = c // 4, c % 4
    b = qq % 2
    jmod = 2 * b2 + qq // 2
    js = [4 * h2 + jmod for h2 in range(NG)]
    return b2, qq, b, js


def _build(causal: bool, n_iter: int = 1):
    import concourse.bass as bass
    import concourse.mybir as mybir
    from concourse import bacc, tile

    F32 = mybir.dt.float32
    F16 = mybir.dt.float16
    AF = mybir.ActivationFunctionType
    MUL = mybir.AluOpType.mult
    ADD = mybir.AluOpType.add

    nc = bacc.Bacc("TRN2", target_bir_lowering=False, debug=False)
    ktp = nc.declare_dram_parameter("ktp", [P, 4 * S], F16, isOutput=False)
    qtp = nc.declare_dram_parameter("qtp", [P, 4 * S], F16, isOutput=False)
    vap = nc.declare_dram_parameter("vap", [P, NT * 4 * P], F16, isOutput=False)
    wtp = nc.declare_dram_parameter("wtp", [P, 16 * F], F16, isOutput=False)
    bop = nc.declare_dram_parameter("bop", [P, 8], F32, isOutput=False)
    mkp = nc.declare_dram_parameter("mkp", [P, P], F16, isOutput=False)
    out = nc.declare_dram_parameter("o", [F, 256], F32, isOutput=True)

    import contextlib
    with tile.TileContext(nc) as tc:
        loop_ctx = tc.For_i(0, n_iter, 1, hint_engines=(
            mybir.EngineType.PE, mybir.EngineType.DVE, mybir.EngineType.Activation,
            mybir.EngineType.SP, mybir.EngineType.Pool,
        )) if n_iter > 1 else contextlib.nullcontext()
        with loop_ctx, \
             tc.tile_pool(name="cst", bufs=1) as cst, \
             tc.tile_pool(name="expp", bufs=6) as expp, \
             tc.tile_pool(name="work", bufs=3) as wkp, \
             tc.tile_pool(name="stps", bufs=2, space="PSUM") as stps, \
             tc.tile_pool(name="rbps", bufs=2, space="PSUM") as rbps, \
             tc.tile_pool(name="xtps", bufs=1, space="PSUM") as xtps:

            kt_sb = cst.tile([P, 4, S], F16)
            qt_sb = cst.tile([P, 4, S], F16)
            va_sb = cst.tile([P, NT, 4 * P], F16)
            wt_sb = cst.tile([P, 16, F], F16)
            bo_sb = cst.tile([P, 8], F32)
            mk_sb = cst.tile([P, P], F16)
            on_sb = cst.tile([P, 32], F16)
            recipb = cst.tile([P, 2, 512], F16)
            xs_sb = cst.tile([P, 4, S], F16)
            nc.sync.dma_start(kt_sb[:, 0, :], ktp[:, 0:S])
            nc.sync.dma_start(qt_sb[:, 0, :], qtp[:, 0:S])
            nc.sync.dma_start(mk_sb[:], mkp[:])
            nc.sync.dma_start(va_sb[:], vap[:])
            nc.sync.dma_start(bo_sb[:], bop[:])
            for qd in range(1, 4):
                nc.sync.dma_start(kt_sb[:, qd, :], ktp[:, S * qd:S * (qd + 1)])
                nc.sync.dma_start(qt_sb[:, qd, :], qtp[:, S * qd:S * (qd + 1)])
            for fd in range(4):
                nc.sync.dma_start(wt_sb[:, 4 * fd:4 * (fd + 1), :],
                                  wtp[:, 4 * F * fd:4 * F * (fd + 1)])
            nc.vector.memset(on_sb[:], 1.0)

            for q in range(4):
                xtb = xtps.tile([P, 2, 512], F32, tag="xt")
                for pair in range(2):
                    for t in range(NT):
                        for j2 in range(2):
                            j = 2 * pair + j2
                            g = 4 * q + j
                            st = stps.tile([P, S], F32, tag="st")
                            t0 = 128 * t if causal else 0
                            # scores, per 512-col chunk (PSUM bank)
                            for c in range(2):
                                lo = max(t0, 512 * c)
                                if lo >= 512 * (c + 1):
                                    continue
                                nc.tensor.matmul(
                                    st[:, lo:512 * (c + 1)],
                                    kt_sb[32 * j:32 * j + 16, q, 128 * t:128 * (t + 1)],
                                    qt_sb[32 * j:32 * j + 16, q, lo:512 * (c + 1)],
                                    start=True, stop=True,
                                    tile_position=(32 * j, 0))
                            expt = expp.tile([P, S], F16, tag="expt")
                            nc.scalar.activation(expt[:, t0:], st[:, t0:], AF.Exp)
                            if causal:
                                nc.vector.tensor_tensor(
                                    out=expt[:, t0:t0 + P], in0=expt[:, t0:t0 + P],
                                    in1=mk_sb[:], op=MUL)
                            # attn @ [v|ones|zeros], col-tiled into shared bank
                            for c in range(2):
                                lo = max(t0, 512 * c)
                                if lo >= 512 * (c + 1):
                                    continue
                                last_t = 3 if (causal and c == 0) else NT - 1
                                nc.tensor.matmul(
                                    xtb[32 * j:32 * (j + 1), c, lo - 512 * c:],
                                    va_sb[:, t, 32 * g:32 * (g + 1)],
                                    expt[:, lo:512 * (c + 1)],
                                    start=(t == 0), stop=(t == last_t),
                                    skip_group_check=True,
                                    tile_position=(0, 32 * j))
                for c in range(2):
                    recip_all = wkp.tile([P, 512], F16, tag="recip")
                    with nc.allow_low_precision(reason="fp16 softmax recip"):
                        nc.vector.reciprocal(recip_all[:], xtb[:, c, :])
                    # PE broadcast: denom-recip row 32j -> whole 32-block
                    rb_ps = rbps.tile([P, 512], F32, tag="rb")
                    for j in range(4):
                        nc.tensor.matmul(
                            rb_ps[32 * j:32 * (j + 1), :],
                            on_sb[32 * j:32 * j + 1, :],
                            recip_all[32 * j:32 * j + 1, :],
                            start=True, stop=True,
                            tile_position=(32 * j, 32 * j))
                    nc.vector.tensor_scalar(
                        out=recipb[:, c, :], in0=rb_ps[:],
                        scalar1=0.0, scalar2=None, op0=ADD)
                    nc.vector.tensor_tensor(
                        out=xs_sb[:, q, 512 * c:512 * (c + 1)],
                        in0=xtb[:, c, :], in1=recipb[:, c, :], op=MUL)


            # output linear: out^T[o, r] = sum_{q,m} wtp_(q,m)^T @ xs[:, q, m::4]
            for ot in range(8):
                ps = stps.tile([P, 256], F32, tag="st")
                for q in range(4):
                    for m in range(4):
                        nc.tensor.matmul(
                            ps[:], wt_sb[:, 4 * q + m, 128 * ot:128 * (ot + 1)],
                            xs_sb[:, q, m:S:4],
                            start=(q == 0 and m == 0), stop=(q == 3 and m == 3))
                ot_sb = wkp.tile([P, 256], F32, tag="ot")
                nc.vector.tensor_scalar(
                    out=ot_sb[:], in0=ps[:], scalar1=bo_sb[:, ot:ot + 1],
                    scalar2=None, op0=ADD)
                nc.sync.dma_start(out=out[128 * ot:128 * (ot + 1), :], in_=ot_sb[:])
    nc.compile()
    return nc


_NC_CACHE = {}


def _get_nc(causal: bool, n_iter: int = 1):
    key = (causal, n_iter)
    if key not in _NC_CACHE:
        _NC_CACHE[key] = _build(causal, n_iter)
    return _NC_CACHE[key]


def _shard_inputs(q, k, v, W_out, b_out):
    """Build the 8 per-core input maps."""
    mk = (np.arange(P)[None, :] >= np.arange(P)[:, None]).astype(np.float16)

    in_maps = []
    for c in range(NCORES):
        _, _, b, js = _core_groups(c)
        cols = np.concatenate([j * H + np.arange(H) for j in js])  # [NG*H]
        kv = k[b][:, cols]          # [S, 256] cols grouped by local g'
        qv = q[b][:, cols] * 0.25
        vv = v[b][:, cols]

        ktm = np.zeros((P, 4 * S), np.float16)
        qtm = np.zeros((P, 4 * S), np.float16)
        for g2 in range(NG):
            qq_, j = divmod(g2, 4)
            rows = slice(32 * j, 32 * j + 16)
            csl = slice(S * qq_, S * (qq_ + 1))
            ktm[rows, csl] = kv[:, 16 * g2:16 * (g2 + 1)].T.astype(np.float16)
            qtm[rows, csl] = qv[:, 16 * g2:16 * (g2 + 1)].T.astype(np.float16)

        vam = np.zeros((P, NT, 4 * P), np.float16)
        for t in range(NT):
            blk = vv[128 * t:128 * (t + 1)].reshape(P, NG, H)  # [p, g', h]
            for g2 in range(NG):
                vam[:, t, 32 * g2] = 1.0
                vam[:, t, 32 * g2 + 1:32 * g2 + 17] = blk[:, g2].astype(np.float16)

        wtm = np.zeros((P, 16 * F), np.float16)
        for q4 in range(4):
            for m in range(4):
                ft = 4 * q4 + m
                for j in range(4):
                    g2 = 4 * q4 + j
                    fidx = 64 * g2 + 16 * m + np.arange(H)
                    wtm[32 * j + 1:32 * j + 17, F * ft:F * (ft + 1)] = \
                        W_out[:, fidx].T.astype(np.float16)

        bom = np.ascontiguousarray(
            b_out.astype(np.float32).reshape(8, P).T)  # [p, ot]

        in_maps.append({
            "ktp": ktm, "qtp": qtm,
            "vap": vam.reshape(P, NT * 4 * P),
            "wtp": wtm, "bop": bom, "mkp": mk,
        })
    return in_maps


def _unshard(outs):
    full = np.empty((B, S, F), np.float32)
    for c in range(NCORES):
        b2, qq, _, _ = _core_groups(c)
        full[b2, 256 * qq:256 * (qq + 1), :] = outs[c].T
    return full


def _numpy_core(in_map, causal=True):
    """Numpy emulation of the device program (for host-logic validation)."""
    ktm = in_map["ktp"].astype(np.float32)
    qtm = in_map["qtp"].astype(np.float32)
    vam = in_map["vap"].reshape(P, NT, 4 * P).astype(np.float32)
    wtm = in_map["wtp"].astype(np.float32)
    bom = in_map["bop"]

    xs = np.zeros((P, 4, S), np.float32)
    for q4 in range(4):
        xt = np.zeros((P, S), np.float32)
        for j in range(4):
            g2 = 4 * q4 + j
            kt = ktm[32 * j:32 * j + 16, S * q4:S * (q4 + 1)]
            qt = qtm[32 * j:32 * j + 16, S * q4:S * (q4 + 1)]
            st = kt.T @ qt                      # [s2, s1]
            e = np.exp(st).astype(np.float16).astype(np.float32)
            if causal:
                s2i, s1i = np.mgrid[0:S, 0:S]
                e = np.where(s1i >= s2i, e, 0.0)
            va = np.concatenate([vam[:, t, :] for t in range(NT)], axis=0)  # [S, 4P]
            xt[32 * j:32 * (j + 1), :] = va[:, 32 * g2:32 * (g2 + 1)].T @ e
        recip_all = np.float16(1.0) / xt.astype(np.float16)
        recipb = np.zeros((P, S), np.float16)
        for j in range(4):
            recipb[32 * j:32 * (j + 1)] = recip_all[32 * j]
        xs[:, q4, :] = (xt * recipb.astype(np.float32)).astype(np.float16)

    oT = np.zeros((F, 256), np.float32)
    for ot in range(8):
        acc = np.zeros((P, 256), np.float32)
        for q4 in range(4):
            for m in range(4):
                lhsT = wtm[:, F * (4 * q4 + m) + 128 * ot:F * (4 * q4 + m) + 128 * (ot + 1)]
                rhs = xs[:, q4, m::4]
                acc += lhsT.T @ rhs
        oT[128 * ot:128 * (ot + 1)] = acc + bom[:, ot][:, None]
    return oT


def kernel(q, k, v, W_out, b_out, apply_mask, _mock=False):
    q = np.asarray(q, np.float32)
    k = np.asarray(k, np.float32)
    v = np.asarray(v, np.float32)
    W_out = np.asarray(W_out, np.float32)
    b_out = np.asarray(b_out, np.float32)
    causal = bool(int(np.asarray(apply_mask)))
    in_maps = _shard_inputs(q, k, v, W_out, b_out)
    if _mock:
        outs = [_numpy_core(m, causal) for m in in_maps]
        return _unshard(outs)
    from concourse.bass_utils import run_bass_kernel_spmd
    nc = _get_nc(causal)
    res = run_bass_kernel_spmd(nc, in_maps, core_ids=list(range(NCORES)))
    return _unshard([r["o"] for r in res.results])
